# revision 23
# baseline (speedup 1.0000x reference)
"""Trainium2 Bass kernel: 8-layer ternary (BitNet-1.58) dense transformer.

Model (per reference):
    h = embed[input_ids]                                  # (B=2, S=1024, H=2048)
    8x: y = h @ ternary(W_l)^T + b_l ; h = LN(y + h)*g+b  # H=2048
    h = LN(h)*final_g + final_b
    logits = h @ ternary(head_W)^T                        # (B, S, V=32000)

Sharding over 8 NeuronCores:
  - Layers: data-parallel over the 2048 tokens (256 tokens/core). Each core
    streams the full ternary layer weights (fp8, exact); no collectives.
  - Head: 8-way tensor-parallel over vocab (4000 vocab rows/core). Final
    hidden states are exchanged SBUF-to-SBUF with direct remote DMA
    broadcasts (XOR-relative dests, one slot per peer), overlapped with the
    core's own-token head blocks; sender identity travels as a one-hot tag
    and the host unscrambles the row blocks.

Engine assignment per layer step (128-token tile):
  TensorE: 16 bf16 transposes (bf16 identity -> 1 cyc/row) + 64 matmuls
           (bf16 stationary hT x fp8 ternary moving weights), kt-major.
  VectorE: fused residual-add+row-sum (scalar_tensor_tensor) + LN stats
           + the (z-mean)*rstd normalize (writes bf16 state).
  PoolE:   Square with accum (sum-of-squares) from SBUF.
  ScalarE: quarter-granular PSUM->SBUF transpose-cast (ternary scale
           folded) + Sqrt.
The emission order software-pipelines steps: each step's transposes+casts
are emitted during the previous step so the first matmul never waits on a
full-tile cast.
"""

import os
import sys

import numpy as np

try:
    import concourse.bass as bass
except ImportError:  # grading container should have it on sys.path already
    sys.path.insert(0, "/opt/trn_rl_repo")
    import concourse.bass as bass

import ml_dtypes
import concourse.mybir as mybir
import concourse.tile as tile
from concourse import bacc
from concourse.bass_utils import run_bass_kernel_spmd
from contextlib import ExitStack

F32 = mybir.dt.float32
BF16 = mybir.dt.bfloat16
FP8 = mybir.dt.float8e4
AX = mybir.AxisListType
OP = mybir.AluOpType
AF = mybir.ActivationFunctionType
EPS = 1e-5

# Full-size problem config (B=2, S=1024 -> 2048 tokens).
CFG_FULL = dict(L=8, H=2048, NTOK=2048, NC=8, TT=2, VS=4000, CH=512, QV=1000, NV=500)


def build_nc(cfg, scales, head_scale, trivial):
    L, H, NTOK, NC, TT = cfg["L"], cfg["H"], cfg["NTOK"], cfg["NC"], cfg["TT"]
    VS, CH, QV, NV = cfg["VS"], cfg["CH"], cfg["QV"], cfg["NV"]
    KT = H // 128
    NCH = H // CH
    NQ = VS // QV
    NVQ = QV // NV
    SENDW = H + 8  # hT payload + one-hot sender tag
    assert NTOK == NC * TT * 128

    nc = bacc.Bacc("TRN2", target_bir_lowering=False, debug=False, num_devices=NC)
    h0 = nc.declare_dram_parameter("h0", [TT, 128, H], BF16, isOutput=False)
    w_ = nc.declare_dram_parameter("w", [L, KT, 128, H], FP8, isOutput=False)
    hw_ = nc.declare_dram_parameter("hw", [KT, 128, VS], FP8, isOutput=False)
    mid_ = nc.declare_dram_parameter("mid", [128, 8], BF16, isOutput=False)
    ident_d = nc.declare_dram_parameter("ident", [128, 128], BF16, isOutput=False)
    eps_d = nc.declare_dram_parameter("eps", [128, 1], F32, isOutput=False)
    if not trivial:
        lng = nc.declare_dram_parameter("lng", [L, H], BF16, isOutput=False)
        lnb = nc.declare_dram_parameter("lnb", [L, H], BF16, isOutput=False)
        lbias = nc.declare_dram_parameter("lbias", [L, H], BF16, isOutput=False)
        fing = nc.declare_dram_parameter("fing", [H], BF16, isOutput=False)
        finb = nc.declare_dram_parameter("finb", [H], BF16, isOutput=False)
    # out rows are BLOCK-indexed: block b = slot j * TT + t; slot 0 = own
    # tokens, slots 1..7 = XOR-peers (host maps via tags).
    out = nc.declare_dram_parameter("out", [NTOK, VS], F32, isOutput=True)
    tags_d = nc.declare_dram_parameter("tags", [TT, 128, NC - 1, 8], BF16,
                                       isOutput=True)

    rsem = [nc.alloc_semaphore(f"rsem{t}") for t in range(TT)]
    lsem = nc.alloc_semaphore("lsem")

    with tile.TileContext(nc) as tc:
        with ExitStack() as ctx0:
            consts = ctx0.enter_context(tc.tile_pool(name="consts", bufs=1))
            sendp = ctx0.enter_context(tc.tile_pool(name="hTsend", bufs=TT))
            remp = ctx0.enter_context(tc.tile_pool(name="hTrem", bufs=TT))

            # stale-semaphore guard for repeated executions; the first clear
            # is gated (post-scheduling) on the prelude kernel barrier so a
            # late-launching core cannot wipe peers' arrival increments.
            clear0 = nc.gpsimd.sem_clear(lsem)
            for t in range(TT):
                nc.gpsimd.sem_clear(rsem[t])

            ident = consts.tile([128, 128], BF16)
            nc.sync.dma_start(ident[:], ident_d[:])
            eps_t = consts.tile([128, 1], F32)
            nc.sync.dma_start(eps_t[:], eps_d[:])
            mid_t = consts.tile([128, 8], BF16)
            nc.sync.dma_start(mid_t[:], mid_[:])

            hTsend = [sendp.tile([128, SENDW], BF16, name=f"hTsend{t}")
                      for t in range(TT)]
            hTrem = [remp.tile([128, NC - 1, SENDW], BF16, name=f"hTrem{t}")
                     for t in range(TT)]

            gate_insts = []  # (tile t, instruction) to gate on rsem[t] arrival

            with ExitStack() as ctxA:
                state = ctxA.enter_context(tc.tile_pool(name="state", bufs=3))
                zp = ctxA.enter_context(tc.tile_pool(name="z", bufs=1))
                sqp = ctxA.enter_context(tc.tile_pool(name="sq", bufs=2))
                hTp = ctxA.enter_context(tc.tile_pool(name="hT", bufs=2))
                wp = ctxA.enter_context(tc.tile_pool(name="w", bufs=2))
                smp = ctxA.enter_context(tc.tile_pool(name="small", bufs=16))
                psT = ctxA.enter_context(
                    tc.tile_pool(name="psT", bufs=4, space="PSUM")
                )
                psY = ctxA.enter_context(
                    tc.tile_pool(name="psY", bufs=4, space="PSUM")
                )
                if not trivial:
                    gbp = ctxA.enter_context(tc.tile_pool(name="gb", bufs=2))

                h_cur = []
                for t in range(TT):
                    st = state.tile([128, H], BF16, name=f"hinit{t}", tag="state")
                    nc.sync.dma_start(st[:], h0[t])
                    h_cur.append(st)

                w_tiles = {}
                KH = KT // 2  # k-tiles per half-layer weight tile

                def load_weights(l):
                    for hf in range(2):
                        wt = wp.tile([128, KH, H], FP8, tag="w", name=f"w{l}_{hf}")
                        nc.sync.dma_start(
                            wt[:],
                            w_[l, hf * KH : (hf + 1) * KH].rearrange("k p o -> p k o"),
                        )
                        w_tiles[(l, hf)] = wt

                gb_tiles = {}

                def load_gb(l):
                    g_t = gbp.tile([128, H], BF16, tag="g", name=f"g{l}")
                    nc.sync.dma_start(g_t[:], lng[l][None, :].to_broadcast((128, H)))
                    b_t = gbp.tile([128, H], BF16, tag="b", name=f"b{l}")
                    nc.sync.dma_start(b_t[:], lnb[l][None, :].to_broadcast((128, H)))
                    bias_t = gbp.tile([128, H], BF16, tag="bias", name=f"bias{l}")
                    nc.sync.dma_start(
                        bias_t[:], lbias[l][None, :].to_broadcast((128, H))
                    )
                    gb_tiles[l] = (g_t, b_t, bias_t)

                def front(l, t, dst=None, scale=None):
                    """Transpose h_cur[t] quarter-wise into PSUM and cast+scale
                    into an SBUF hT tile (bf16). Returns the hT tile."""
                    if scale is None:
                        scale = scales[l]
                    if dst is None:
                        dst = hTp.tile([128, H], BF16, tag="hT", name=f"hT{l}_{t}")
                    src = h_cur[t]
                    for q in range(NCH):
                        pT = psT.tile([128, CH], BF16, tag="psT",
                                      name=f"psT{l}_{t}_{q}")
                        for j in range(CH // 128):
                            kt = q * (CH // 128) + j
                            nc.tensor.transpose(
                                pT[:, j * 128 : (j + 1) * 128],
                                src[:, kt * 128 : (kt + 1) * 128],
                                ident[:],
                            )
                        nc.scalar.activation(
                            dst[:, q * CH : (q + 1) * CH], pT[:], AF.Copy,
                            scale=float(scale),
                        )
                    return dst

                def mm_step(l, t, hTt):
                    ps = [
                        psY.tile([128, CH], F32, tag="psY", name=f"ps{l}_{t}_{i}")
                        for i in range(NCH)
                    ]
                    for kt in range(KT):
                        wt = w_tiles[(l, kt // KH)]
                        for i in range(NCH):
                            nc.tensor.matmul(
                                ps[i][:],
                                lhsT=hTt[:, kt * 128 : (kt + 1) * 128],
                                rhs=wt[:, kt % KH, i * CH : (i + 1) * CH],
                                start=(kt == 0),
                                stop=(kt == KT - 1),
                            )
                    return ps

                def reduce_step(l, t, ps):
                    """residual add + per-chunk row-sums S4 / sq-sums SS4."""
                    resid = h_cur[t]
                    if not trivial:
                        _, _, bias_t = gb_tiles[l]
                        hb = zp.tile([128, H], BF16, tag="hb", name=f"hb{l}_{t}")
                        nc.gpsimd.tensor_tensor(hb[:], h_cur[t][:], bias_t[:], OP.add)
                        resid = hb
                    z = zp.tile([128, H], F32, tag="z", name=f"z{l}_{t}")
                    S4 = smp.tile([128, NCH], F32, tag="s7", name=f"S4_{l}_{t}")
                    SS4 = smp.tile([128, NCH], F32, tag="s8", name=f"SS4_{l}_{t}")
                    for i in range(NCH):
                        sl = slice(i * CH, (i + 1) * CH)
                        nc.vector.scalar_tensor_tensor(
                            z[:, sl], ps[i][:], 0.0, resid[:, sl],
                            OP.add, OP.add, accum_out=S4[:, i : i + 1],
                        )
                        sq = sqp.tile([128, CH], BF16, tag="sq", name=f"sq{l}_{t}_{i}")
                        nc.scalar.activation(
                            sq[:], z[:, sl], AF.Square,
                            accum_out=SS4[:, i : i + 1],
                        )
                    return z, S4, SS4

                def stats_smalls(S4, SS4, name):
                    S = smp.tile([128, 1], F32, tag="s0", name=f"S{name}")
                    nc.vector.tensor_reduce(S[:], S4[:], axis=AX.X, op=OP.add)
                    SS = smp.tile([128, 1], F32, tag="s1", name=f"SS{name}")
                    nc.vector.tensor_reduce(SS[:], SS4[:], axis=AX.X, op=OP.add)
                    negmean = smp.tile([128, 1], F32, tag="s2", name=f"nm{name}")
                    nc.vector.tensor_scalar_mul(negmean[:], S[:], -1.0 / H)
                    msq = smp.tile([128, 1], F32, tag="s3", name=f"msq{name}")
                    nc.vector.tensor_scalar_mul(msq[:], SS[:], 1.0 / H)
                    var = smp.tile([128, 1], F32, tag="s4", name=f"var{name}")
                    nc.vector.tensor_tensor(var[:], negmean[:], negmean[:], OP.mult)
                    nc.vector.tensor_tensor(var[:], msq[:], var[:], OP.subtract)
                    return negmean, var

                def sqrt_norm(z_src, negmean, var, name, gl=None):
                    std = smp.tile([128, 1], F32, tag="s5", name=f"std{name}")
                    nc.scalar.activation(std[:], var[:], AF.Sqrt, bias=eps_t[:])
                    rstd = smp.tile([128, 1], F32, tag="s6", name=f"rstd{name}")
                    nc.vector.reciprocal(rstd[:], std[:])
                    hn = state.tile([128, H], BF16, tag="state", name=f"h{name}")
                    nc.vector.tensor_scalar(
                        hn[:], z_src[:], negmean[:], rstd[:], OP.add, OP.mult
                    )
                    if gl is not None:
                        g_t, b_t = gl
                        nc.gpsimd.tensor_tensor(hn[:], hn[:], g_t[:], OP.mult)
                        nc.gpsimd.tensor_tensor(hn[:], hn[:], b_t[:], OP.add)
                    return hn

                def fin_stats(t):
                    h8 = h_cur[t]
                    S4 = smp.tile([128, NCH], F32, tag="s7", name=f"S4f{t}")
                    SS4 = smp.tile([128, NCH], F32, tag="s8", name=f"SS4f{t}")
                    for i in range(NCH):
                        sl = slice(i * CH, (i + 1) * CH)
                        nc.vector.tensor_reduce(
                            S4[:, i : i + 1], h8[:, sl], axis=AX.X, op=OP.add
                        )
                        sq = sqp.tile([128, CH], BF16, tag="sq", name=f"sqf{t}_{i}")
                        nc.scalar.activation(
                            sq[:], h8[:, sl], AF.Square,
                            accum_out=SS4[:, i : i + 1],
                        )
                    return h8, S4, SS4

                def send_tile(t):
                    """Broadcast hTsend[t] to the 7 XOR-peers, slot j."""
                    nc.vector.tensor_copy(hTsend[t][:, H : H + 8], mid_t[:])
                    for j in range(1, NC):
                        rd = [None] * 8
                        rd[j] = (0, j)
                        nc.gpsimd.remote_dma_broadcast(
                            hTrem[t][:, j - 1, :],
                            hTsend[t][:],
                            remote_sem=rsem[t],
                            local_sem=lsem,
                            rdests=rd,
                        )
                    nc.gpsimd.trigger_dma(count=None)
                    # tag staging copy; gated on arrival post-scheduling
                    tstg = consts.tile([128, NC - 1, 8], BF16,
                                       name=f"tagstg{t}")
                    tc_i = nc.vector.tensor_copy(
                        tstg[:], hTrem[t][:, :, H : H + 8]
                    )
                    gate_insts.append((t, tc_i))
                    nc.sync.dma_start(tags_d[t], tstg[:])

                # ---- software-pipelined layer loop ----
                # steps: (l, t) for l in 0..L-1, then final-LN handled after.
                steps = [(l, t) for l in range(L) for t in range(TT)]
                load_weights(0)
                if not trivial:
                    load_gb(0)
                # head weights prefetched in head scope below (streamed).
                hT_tiles = {}
                hT_tiles[steps[0]] = front(*steps[0])
                for si, (l, t) in enumerate(steps):
                    if t == 0 and l + 1 < L:
                        load_weights(l + 1)
                        if not trivial:
                            load_gb(l + 1)
                    ps = mm_step(l, t, hT_tiles.pop((l, t)))
                    z, S4, SS4 = reduce_step(l, t, ps)
                    negmean, var = stats_smalls(S4, SS4, f"{l}_{t}")
                    if si + 1 < len(steps):
                        hT_tiles[steps[si + 1]] = front(*steps[si + 1])
                    gl = None
                    if not trivial:
                        g_t, b_t, _ = gb_tiles[l]
                        gl = (g_t, b_t)
                    h_cur[t] = sqrt_norm(z, negmean, var, f"{l}_{t}", gl)

                # ---- final LN + transpose + sends, pipelined with last steps
                fgl = None
                if not trivial:
                    fg = gbp.tile([128, H], BF16, tag="g", name="gfin")
                    nc.sync.dma_start(fg[:], fing[None, :].to_broadcast((128, H)))
                    fb = gbp.tile([128, H], BF16, tag="b", name="bfin")
                    nc.sync.dma_start(fb[:], finb[None, :].to_broadcast((128, H)))
                    fgl = (fg, fb)
                for t in range(TT):
                    h8, S4, SS4 = fin_stats(t)
                    negmean, var = stats_smalls(S4, SS4, f"fin{t}")
                    h_cur[t] = sqrt_norm(h8, negmean, var, f"fin{t}", fgl)
                    front(None, t, dst=hTsend[t], scale=head_scale)
                    send_tile(t)

            # ---- head phase: q-outer streamed weights ----
            with ExitStack() as ctxB:
                wqp = ctxB.enter_context(tc.tile_pool(name="wq", bufs=2))
                outp = ctxB.enter_context(tc.tile_pool(name="outstg", bufs=6))
                psH = ctxB.enter_context(
                    tc.tile_pool(name="psH", bufs=4, space="PSUM")
                )

                first_remote_gated = [False] * TT

                def head_block(src_fn, b, t, q, wq):
                    """one 128-token row block x QV vocab cols"""
                    row0 = b * 128
                    for vi in range(NVQ):
                        p = psH.tile([128, NV], F32, tag="psH",
                                     name=f"ph{b}_{t}_{q}_{vi}")
                        for kt in range(KT):
                            mm = nc.tensor.matmul(
                                p[:],
                                lhsT=src_fn(kt),
                                rhs=wq[:, kt, vi * NV : (vi + 1) * NV],
                                start=(kt == 0),
                                stop=(kt == KT - 1),
                            )
                            if kt == 0 and b >= TT and not first_remote_gated[t]:
                                gate_insts.append((t, mm))
                                first_remote_gated[t] = True
                        o_t = outp.tile([128, NV], F32, tag="ostg",
                                        name=f"o{b}_{q}_{vi}")
                        nc.scalar.copy(o_t[:], p[:])
                        nc.sync.dma_start(
                            out[row0 : row0 + 128,
                                q * QV + vi * NV : q * QV + (vi + 1) * NV],
                            o_t[:],
                        )

                for q in range(NQ):
                    wq = wqp.tile([128, KT, QV], FP8, tag="wq", name=f"wq{q}")
                    nc.sync.dma_start(
                        wq[:],
                        hw_[:, :, q * QV : (q + 1) * QV].rearrange("k p v -> p k v"),
                    )
                    # self blocks first (j=0), then XOR-slot peers
                    for j in range(NC):
                        for t in range(TT):
                            b = j * TT + t
                            if j == 0:
                                src_fn = (
                                    lambda kt, t=t: hTsend[t][
                                        :, kt * 128 : (kt + 1) * 128
                                    ]
                                )
                            else:
                                src_fn = (
                                    lambda kt, t=t, j=j: hTrem[t][
                                        :, j - 1, kt * 128 : (kt + 1) * 128
                                    ]
                                )
                            head_block(src_fn, b, t, q, wq)

    # Post-scheduling semaphore gates. The Tile scheduling sim cannot model
    # cross-core increments, so waits are spliced in after scheduling;
    # finalize's event-semaphore split pass handles multi-wait instructions.
    def splice_wait(ins, sem, value):
        si = ins.sync_info
        waits = list(si.on_wait) if si else []
        ups = list(si.on_update) if si else []
        waits.append(
            mybir.SyncWait(
                sync_type="semaphore", id=sem.num, ant_name=sem.name,
                wait_mode="sem-ge-imm", wait_value=value,
            )
        )
        ins.sync_info = mybir.SyncInfo(on_wait=waits, on_update=ups)

    # 1) first consumer of each tile's remote data waits on arrival
    #    (2 increments per peer per broadcast).
    for t, bi in gate_insts:
        splice_wait(bi.ins, rsem[t], 2 * (NC - 1))
    # 2) the sem clears wait on the prelude kernel-entry barrier (an
    #    AllGather inserted at finalize), absorbing launch skew.
    nc._bir_kernel_barrier_sem_replica_groups.extend([set(range(NC))])
    splice_wait(clear0.ins, nc._bir_kernel_barrier_sem,
                nc.bir_kernel_barrier_sem_inc)

    return nc


def _ternary(wmat):
    """Exact {-1,0,1} ternary tensor + fp32 scale, matching the reference."""
    w = np.asarray(wmat, dtype=np.float32)
    s = np.mean(np.abs(w), dtype=np.float32)
    t = np.clip(np.rint(w / (s + np.float32(1e-8))), -1.0, 1.0).astype(np.float32)
    return t, float(s)


_NC_CACHE = {}
_LAST_RESULTS = None


def kernel(**inputs):
    global _LAST_RESULTS
    cfg = CFG_FULL
    L, H, NTOK, NC, TT, VS = (
        cfg["L"], cfg["H"], cfg["NTOK"], cfg["NC"], cfg["TT"], cfg["VS"],
    )
    KT = H // 128
    TPC = TT * 128  # tokens per core
    BF = ml_dtypes.bfloat16
    E4 = ml_dtypes.float8_e4m3

    ids = np.asarray(inputs["input_ids"]).astype(np.int64).reshape(-1)
    embed = np.asarray(inputs["embed"], dtype=np.float32)
    layer_w = np.asarray(inputs["layer_w"], dtype=np.float32)
    layer_b = np.asarray(inputs["layer_b"], dtype=np.float32)
    ln_g = np.asarray(inputs["ln_g"], dtype=np.float32)
    ln_b = np.asarray(inputs["ln_b"], dtype=np.float32)
    final_g = np.asarray(inputs["final_g"], dtype=np.float32)
    final_b = np.asarray(inputs["final_b"], dtype=np.float32)
    head_w = np.asarray(inputs["head_w"], dtype=np.float32)

    trivial = bool(
        np.all(ln_g == 1.0) and np.all(ln_b == 0.0) and np.all(layer_b == 0.0)
        and np.all(final_g == 1.0) and np.all(final_b == 0.0)
    )

    h0_full = embed[ids]  # [NTOK, H] fp32

    scales = []
    wT = np.empty([L, KT, 128, H], dtype=E4)
    for l in range(L):
        t, s = _ternary(layer_w[l])
        scales.append(s)
        wT[l] = np.ascontiguousarray(t.T).reshape(KT, 128, H).astype(E4)
    th, head_scale = _ternary(head_w)
    headT = np.ascontiguousarray(th.T).astype(E4)  # [H, V]

    key = (id(cfg), tuple(scales), head_scale, trivial)
    if key not in _NC_CACHE:
        _NC_CACHE.clear()
        nc = build_nc(cfg, scales, head_scale, trivial)
        nc.finalize()
        _NC_CACHE[key] = nc
    nc = _NC_CACHE[key]

    common = {
        "w": wT,
        "ident": np.eye(128, dtype=BF),
        "eps": np.full((128, 1), EPS, np.float32),
    }
    if not trivial:
        common.update(
            lng=ln_g.astype(BF),
            lnb=ln_b.astype(BF),
            lbias=layer_b.astype(BF),
            fing=final_g.astype(BF),
            finb=final_b.astype(BF),
        )
    in_maps = []
    for c in range(NC):
        mid = np.zeros((128, 8), dtype=BF)
        mid[:, c] = 1.0
        in_maps.append(
            dict(
                common,
                h0=np.ascontiguousarray(
                    h0_full[c * TPC : (c + 1) * TPC].reshape(TT, 128, H)
                ).astype(BF),
                hw=np.ascontiguousarray(
                    headT[:, c * VS : (c + 1) * VS].reshape(KT, 128, VS)
                ),
                mid=mid,
            )
        )

    trace = bool(int(os.environ.get("TRIKERNEL_TRACE", "0")))
    res = run_bass_kernel_spmd(nc, in_maps, core_ids=list(range(NC)), trace=trace)
    _LAST_RESULTS = res

    # unscramble: core r's row block b = j*TT + t holds tokens of sender s
    # (s = r for j=0, else read from the one-hot tag), vocab shard r.
    full = np.empty((NTOK, NC * VS), dtype=np.float32)
    for r in range(NC):
        o = np.asarray(res.results[r]["out"])  # [NTOK, VS] block-rows
        tags = np.asarray(res.results[r]["tags"]).astype(np.float32)
        for j in range(NC):
            for t in range(TT):
                b = j * TT + t
                if j == 0:
                    s = r
                else:
                    s = int(np.argmax(tags[t, 0, j - 1]))
                full[
                    s * TPC + t * 128 : s * TPC + (t + 1) * 128,
                    r * VS : (r + 1) * VS,
                ] = o[b * 128 : (b + 1) * 128]
    return full.reshape(2, 1024, 32000).astype(np.float32)


# revision 24
# speedup vs baseline: 1.0134x; 1.0134x over previous
"""Trainium2 Bass kernel: 8-layer ternary (BitNet-1.58) dense transformer.

Model (per reference):
    h = embed[input_ids]                                  # (B=2, S=1024, H=2048)
    8x: y = h @ ternary(W_l)^T + b_l ; h = LN(y + h)*g+b  # H=2048
    h = LN(h)*final_g + final_b
    logits = h @ ternary(head_W)^T                        # (B, S, V=32000)

Sharding over 8 NeuronCores:
  - Layers: data-parallel over the 2048 tokens (256 tokens/core). Each core
    streams the full ternary layer weights (fp8, exact); no collectives.
  - Head: 8-way tensor-parallel over vocab (4000 vocab rows/core). Final
    hidden states are exchanged SBUF-to-SBUF with direct remote DMA
    broadcasts (XOR-relative dests, one slot per peer), overlapped with the
    core's own-token head blocks; sender identity travels as a one-hot tag
    and the host unscrambles the row blocks.

Engine assignment per layer step (128-token tile):
  TensorE: 16 bf16 transposes (bf16 identity -> 1 cyc/row) + 64 matmuls
           (bf16 stationary hT x fp8 ternary moving weights), kt-major.
  VectorE: fused residual-add+row-sum (scalar_tensor_tensor) + LN stats
           + the (z-mean)*rstd normalize (writes bf16 state).
  PoolE:   Square with accum (sum-of-squares) from SBUF.
  ScalarE: quarter-granular PSUM->SBUF transpose-cast (ternary scale
           folded) + Sqrt.
The emission order software-pipelines steps: each step's transposes+casts
are emitted during the previous step so the first matmul never waits on a
full-tile cast.
"""

import os
import sys

import numpy as np

try:
    import concourse.bass as bass
except ImportError:  # grading container should have it on sys.path already
    sys.path.insert(0, "/opt/trn_rl_repo")
    import concourse.bass as bass

import ml_dtypes
import concourse.mybir as mybir
import concourse.tile as tile
from concourse import bacc
from concourse.bass_utils import run_bass_kernel_spmd
from contextlib import ExitStack

F32 = mybir.dt.float32
BF16 = mybir.dt.bfloat16
FP8 = mybir.dt.float8e4
AX = mybir.AxisListType
OP = mybir.AluOpType
AF = mybir.ActivationFunctionType
EPS = 1e-5

# Full-size problem config (B=2, S=1024 -> 2048 tokens).
CFG_FULL = dict(L=8, H=2048, NTOK=2048, NC=8, TT=2, VS=4000, CH=512, QV=1000, NV=500)


def build_nc(cfg, scales, head_scale, trivial):
    L, H, NTOK, NC, TT = cfg["L"], cfg["H"], cfg["NTOK"], cfg["NC"], cfg["TT"]
    VS, CH, QV, NV = cfg["VS"], cfg["CH"], cfg["QV"], cfg["NV"]
    KT = H // 128
    NCH = H // CH
    NQ = VS // QV
    NVQ = QV // NV
    SENDW = H + 8  # hT payload + one-hot sender tag
    assert NTOK == NC * TT * 128

    nc = bacc.Bacc("TRN2", target_bir_lowering=False, debug=False, num_devices=NC)
    h0 = nc.declare_dram_parameter("h0", [TT, 128, H], BF16, isOutput=False)
    w_ = nc.declare_dram_parameter("w", [L, KT, 128, H], FP8, isOutput=False)
    hw_ = nc.declare_dram_parameter("hw", [KT, 128, VS], FP8, isOutput=False)
    mid_ = nc.declare_dram_parameter("mid", [128, 8], BF16, isOutput=False)
    ident_d = nc.declare_dram_parameter("ident", [128, 128], BF16, isOutput=False)
    eps_d = nc.declare_dram_parameter("eps", [128, 1], F32, isOutput=False)
    if not trivial:
        lng = nc.declare_dram_parameter("lng", [L, H], BF16, isOutput=False)
        lnb = nc.declare_dram_parameter("lnb", [L, H], BF16, isOutput=False)
        lbias = nc.declare_dram_parameter("lbias", [L, H], BF16, isOutput=False)
        fing = nc.declare_dram_parameter("fing", [H], BF16, isOutput=False)
        finb = nc.declare_dram_parameter("finb", [H], BF16, isOutput=False)
    # out rows are BLOCK-indexed: block b = slot j * TT + t; slot 0 = own
    # tokens, slots 1..7 = XOR-peers (host maps via tags).
    out = nc.declare_dram_parameter("out", [NTOK, VS], F32, isOutput=True)
    tags_d = nc.declare_dram_parameter("tags", [TT, 128, NC - 1, 8], BF16,
                                       isOutput=True)

    rsem = [nc.alloc_semaphore(f"rsem{t}") for t in range(TT)]
    lsem = nc.alloc_semaphore("lsem")

    with tile.TileContext(nc) as tc:
        with ExitStack() as ctx0:
            consts = ctx0.enter_context(tc.tile_pool(name="consts", bufs=1))
            sendp = ctx0.enter_context(tc.tile_pool(name="hTsend", bufs=TT))
            remp = ctx0.enter_context(tc.tile_pool(name="hTrem", bufs=TT))

            # stale-semaphore guard for repeated executions; the first clear
            # is gated (post-scheduling) on the prelude kernel barrier so a
            # late-launching core cannot wipe peers' arrival increments.
            clear0 = nc.gpsimd.sem_clear(lsem)
            for t in range(TT):
                nc.gpsimd.sem_clear(rsem[t])

            ident = consts.tile([128, 128], BF16)
            nc.sync.dma_start(ident[:], ident_d[:])
            eps_t = consts.tile([128, 1], F32)
            nc.sync.dma_start(eps_t[:], eps_d[:])
            mid_t = consts.tile([128, 8], BF16)
            nc.sync.dma_start(mid_t[:], mid_[:])

            hTsend = [sendp.tile([128, SENDW], BF16, name=f"hTsend{t}")
                      for t in range(TT)]
            hTrem = [remp.tile([128, NC - 1, SENDW], BF16, name=f"hTrem{t}")
                     for t in range(TT)]

            gate_insts = []  # (tile t, instruction) to gate on rsem[t] arrival

            with ExitStack() as ctxA:
                state = ctxA.enter_context(tc.tile_pool(name="state", bufs=3))
                zp = ctxA.enter_context(tc.tile_pool(name="z", bufs=1))
                sqp = ctxA.enter_context(tc.tile_pool(name="sq", bufs=2))
                hTp = ctxA.enter_context(tc.tile_pool(name="hT", bufs=2))
                wp = ctxA.enter_context(tc.tile_pool(name="w", bufs=2))
                smp = ctxA.enter_context(tc.tile_pool(name="small", bufs=16))
                psT = ctxA.enter_context(
                    tc.tile_pool(name="psT", bufs=4, space="PSUM")
                )
                psY = ctxA.enter_context(
                    tc.tile_pool(name="psY", bufs=4, space="PSUM")
                )
                if not trivial:
                    gbp = ctxA.enter_context(tc.tile_pool(name="gb", bufs=2))

                h_cur = []
                for t in range(TT):
                    st = state.tile([128, H], BF16, name=f"hinit{t}", tag="state")
                    nc.sync.dma_start(st[:], h0[t])
                    h_cur.append(st)

                w_tiles = {}
                KH = KT // 2  # k-tiles per half-layer weight tile

                def load_weights(l):
                    for hf in range(2):
                        wt = wp.tile([128, KH, H], FP8, tag="w", name=f"w{l}_{hf}")
                        nc.sync.dma_start(
                            wt[:],
                            w_[l, hf * KH : (hf + 1) * KH].rearrange("k p o -> p k o"),
                        )
                        w_tiles[(l, hf)] = wt

                gb_tiles = {}

                def load_gb(l):
                    g_t = gbp.tile([128, H], BF16, tag="g", name=f"g{l}")
                    nc.sync.dma_start(g_t[:], lng[l][None, :].to_broadcast((128, H)))
                    b_t = gbp.tile([128, H], BF16, tag="b", name=f"b{l}")
                    nc.sync.dma_start(b_t[:], lnb[l][None, :].to_broadcast((128, H)))
                    bias_t = gbp.tile([128, H], BF16, tag="bias", name=f"bias{l}")
                    nc.sync.dma_start(
                        bias_t[:], lbias[l][None, :].to_broadcast((128, H))
                    )
                    gb_tiles[l] = (g_t, b_t, bias_t)

                def front(l, t, dst=None, scale=None):
                    """Transpose h_cur[t] quarter-wise into PSUM and cast+scale
                    into an SBUF hT tile (bf16). Returns the hT tile."""
                    if scale is None:
                        scale = scales[l]
                    if dst is None:
                        dst = hTp.tile([128, H], BF16, tag="hT", name=f"hT{l}_{t}")
                    src = h_cur[t]
                    for q in range(NCH):
                        pT = psT.tile([128, CH], BF16, tag="psT",
                                      name=f"psT{l}_{t}_{q}")
                        for j in range(CH // 128):
                            kt = q * (CH // 128) + j
                            nc.tensor.transpose(
                                pT[:, j * 128 : (j + 1) * 128],
                                src[:, kt * 128 : (kt + 1) * 128],
                                ident[:],
                            )
                        nc.scalar.activation(
                            dst[:, q * CH : (q + 1) * CH], pT[:], AF.Copy,
                            scale=float(scale),
                        )
                    return dst

                def mm_step(l, t, hTt):
                    ps = [
                        psY.tile([128, CH], F32, tag="psY", name=f"ps{l}_{t}_{i}")
                        for i in range(NCH)
                    ]
                    for kt in range(KT):
                        wt = w_tiles[(l, kt // KH)]
                        for i in range(NCH):
                            nc.tensor.matmul(
                                ps[i][:],
                                lhsT=hTt[:, kt * 128 : (kt + 1) * 128],
                                rhs=wt[:, kt % KH, i * CH : (i + 1) * CH],
                                start=(kt == 0),
                                stop=(kt == KT - 1),
                            )
                    return ps

                def reduce_step(l, t, ps):
                    """residual add + per-chunk row-sums S4 / sq-sums SS4."""
                    resid = h_cur[t]
                    if not trivial:
                        _, _, bias_t = gb_tiles[l]
                        hb = zp.tile([128, H], BF16, tag="hb", name=f"hb{l}_{t}")
                        nc.gpsimd.tensor_tensor(hb[:], h_cur[t][:], bias_t[:], OP.add)
                        resid = hb
                    z = zp.tile([128, H], F32, tag="z", name=f"z{l}_{t}")
                    S4 = smp.tile([128, NCH], F32, tag="s7", name=f"S4_{l}_{t}")
                    SS4 = smp.tile([128, NCH], F32, tag="s8", name=f"SS4_{l}_{t}")
                    for i in range(NCH):
                        sl = slice(i * CH, (i + 1) * CH)
                        nc.vector.scalar_tensor_tensor(
                            z[:, sl], ps[i][:], 0.0, resid[:, sl],
                            OP.add, OP.add, accum_out=S4[:, i : i + 1],
                        )
                        sq = sqp.tile([128, CH], BF16, tag="sq", name=f"sq{l}_{t}_{i}")
                        nc.scalar.activation(
                            sq[:], z[:, sl], AF.Square,
                            accum_out=SS4[:, i : i + 1],
                        )
                    return z, S4, SS4

                def stats_smalls(S4, SS4, name):
                    S = smp.tile([128, 1], F32, tag="s0", name=f"S{name}")
                    nc.vector.tensor_reduce(S[:], S4[:], axis=AX.X, op=OP.add)
                    SS = smp.tile([128, 1], F32, tag="s1", name=f"SS{name}")
                    nc.vector.tensor_reduce(SS[:], SS4[:], axis=AX.X, op=OP.add)
                    negmean = smp.tile([128, 1], F32, tag="s2", name=f"nm{name}")
                    nc.vector.tensor_scalar_mul(negmean[:], S[:], -1.0 / H)
                    msq = smp.tile([128, 1], F32, tag="s3", name=f"msq{name}")
                    nc.vector.tensor_scalar_mul(msq[:], SS[:], 1.0 / H)
                    var = smp.tile([128, 1], F32, tag="s4", name=f"var{name}")
                    nc.vector.tensor_tensor(var[:], negmean[:], negmean[:], OP.mult)
                    nc.vector.tensor_tensor(var[:], msq[:], var[:], OP.subtract)
                    return negmean, var

                def sqrt_norm(z_src, negmean, var, name, gl=None):
                    std = smp.tile([128, 1], F32, tag="s5", name=f"std{name}")
                    nc.scalar.activation(std[:], var[:], AF.Sqrt, bias=eps_t[:])
                    rstd = smp.tile([128, 1], F32, tag="s6", name=f"rstd{name}")
                    nc.vector.reciprocal(rstd[:], std[:])
                    hn = state.tile([128, H], BF16, tag="state", name=f"h{name}")
                    nc.vector.tensor_scalar(
                        hn[:], z_src[:], negmean[:], rstd[:], OP.add, OP.mult
                    )
                    if gl is not None:
                        g_t, b_t = gl
                        nc.gpsimd.tensor_tensor(hn[:], hn[:], g_t[:], OP.mult)
                        nc.gpsimd.tensor_tensor(hn[:], hn[:], b_t[:], OP.add)
                    return hn

                def fin_stats(t):
                    h8 = h_cur[t]
                    S4 = smp.tile([128, NCH], F32, tag="s7", name=f"S4f{t}")
                    SS4 = smp.tile([128, NCH], F32, tag="s8", name=f"SS4f{t}")
                    for i in range(NCH):
                        sl = slice(i * CH, (i + 1) * CH)
                        nc.vector.tensor_reduce(
                            S4[:, i : i + 1], h8[:, sl], axis=AX.X, op=OP.add
                        )
                        sq = sqp.tile([128, CH], BF16, tag="sq", name=f"sqf{t}_{i}")
                        nc.scalar.activation(
                            sq[:], h8[:, sl], AF.Square,
                            accum_out=SS4[:, i : i + 1],
                        )
                    return h8, S4, SS4

                def send_tile(t):
                    """Broadcast hTsend[t] to the 7 XOR-peers, slot j."""
                    nc.vector.tensor_copy(hTsend[t][:, H : H + 8], mid_t[:])
                    for j in range(1, NC):
                        rd = [None] * 8
                        rd[j] = (0, j)
                        nc.gpsimd.remote_dma_broadcast(
                            hTrem[t][:, j - 1, :],
                            hTsend[t][:],
                            remote_sem=rsem[t],
                            local_sem=lsem,
                            rdests=rd,
                        )
                    nc.gpsimd.trigger_dma(count=None)
                    # tag staging copy; gated on arrival post-scheduling
                    tstg = consts.tile([128, NC - 1, 8], BF16,
                                       name=f"tagstg{t}")
                    tc_i = nc.vector.tensor_copy(
                        tstg[:], hTrem[t][:, :, H : H + 8]
                    )
                    gate_insts.append((t, tc_i))
                    nc.sync.dma_start(tags_d[t], tstg[:])

                # ---- software-pipelined layer loop ----
                # steps: (l, t) for l in 0..L-1, then final-LN handled after.
                steps = [(l, t) for l in range(L) for t in range(TT)]
                load_weights(0)
                if not trivial:
                    load_gb(0)
                # head weights prefetched in head scope below (streamed).
                hT_tiles = {}
                hT_tiles[steps[0]] = front(*steps[0])
                for si, (l, t) in enumerate(steps):
                    if t == 0 and l + 1 < L:
                        load_weights(l + 1)
                        if not trivial:
                            load_gb(l + 1)
                    ps = mm_step(l, t, hT_tiles.pop((l, t)))
                    z, S4, SS4 = reduce_step(l, t, ps)
                    negmean, var = stats_smalls(S4, SS4, f"{l}_{t}")
                    if si + 1 < len(steps):
                        hT_tiles[steps[si + 1]] = front(*steps[si + 1])
                    gl = None
                    if not trivial:
                        g_t, b_t, _ = gb_tiles[l]
                        gl = (g_t, b_t)
                    h_cur[t] = sqrt_norm(z, negmean, var, f"{l}_{t}", gl)

                # ---- final LN + transpose + sends, pipelined with last steps
                fgl = None
                if not trivial:
                    fg = gbp.tile([128, H], BF16, tag="g", name="gfin")
                    nc.sync.dma_start(fg[:], fing[None, :].to_broadcast((128, H)))
                    fb = gbp.tile([128, H], BF16, tag="b", name="bfin")
                    nc.sync.dma_start(fb[:], finb[None, :].to_broadcast((128, H)))
                    fgl = (fg, fb)
                for t in range(TT):
                    h8, S4, SS4 = fin_stats(t)
                    negmean, var = stats_smalls(S4, SS4, f"fin{t}")
                    h_cur[t] = sqrt_norm(h8, negmean, var, f"fin{t}", fgl)
                    front(None, t, dst=hTsend[t], scale=head_scale)
                    send_tile(t)

            # ---- head phase: q-outer streamed weights ----
            with ExitStack() as ctxB:
                wqp = ctxB.enter_context(tc.tile_pool(name="wq", bufs=2))
                outp = ctxB.enter_context(tc.tile_pool(name="outstg", bufs=6))
                psH = ctxB.enter_context(
                    tc.tile_pool(name="psH", bufs=4, space="PSUM")
                )

                first_remote_gated = [False] * TT

                def head_block(src_fn, b, t, q, wq):
                    """one 128-token row block x QV vocab cols"""
                    row0 = b * 128
                    for vi in range(NVQ):
                        p = psH.tile([128, NV], F32, tag="psH",
                                     name=f"ph{b}_{t}_{q}_{vi}")
                        for kt in range(KT):
                            mm = nc.tensor.matmul(
                                p[:],
                                lhsT=src_fn(kt),
                                rhs=wq[:, kt, vi * NV : (vi + 1) * NV],
                                start=(kt == 0),
                                stop=(kt == KT - 1),
                            )
                            if kt == 0 and b >= TT and not first_remote_gated[t]:
                                gate_insts.append((t, mm))
                                first_remote_gated[t] = True
                        o_t = outp.tile([128, NV], F32, tag="ostg",
                                        name=f"o{b}_{q}_{vi}")
                        nc.scalar.copy(o_t[:], p[:])
                        nc.sync.dma_start(
                            out[row0 : row0 + 128,
                                q * QV + vi * NV : q * QV + (vi + 1) * NV],
                            o_t[:],
                        )

                # pass 1: all self blocks (ungated) — covers the broadcast
                # flight time; pass 2: remote blocks, weights re-streamed.
                for pass_j in [[0], list(range(1, NC))]:
                    for q in range(NQ):
                        wq = wqp.tile(
                            [128, KT, QV], FP8, tag="wq",
                            name=f"wq{pass_j[0]}_{q}",
                        )
                        nc.sync.dma_start(
                            wq[:],
                            hw_[:, :, q * QV : (q + 1) * QV].rearrange(
                                "k p v -> p k v"
                            ),
                        )
                        for j in pass_j:
                            for t in range(TT):
                                b = j * TT + t
                                if j == 0:
                                    src_fn = (
                                        lambda kt, t=t: hTsend[t][
                                            :, kt * 128 : (kt + 1) * 128
                                        ]
                                    )
                                else:
                                    src_fn = (
                                        lambda kt, t=t, j=j: hTrem[t][
                                            :, j - 1, kt * 128 : (kt + 1) * 128
                                        ]
                                    )
                                head_block(src_fn, b, t, q, wq)

    # Post-scheduling semaphore gates. The Tile scheduling sim cannot model
    # cross-core increments, so waits are spliced in after scheduling;
    # finalize's event-semaphore split pass handles multi-wait instructions.
    def splice_wait(ins, sem, value):
        si = ins.sync_info
        waits = list(si.on_wait) if si else []
        ups = list(si.on_update) if si else []
        waits.append(
            mybir.SyncWait(
                sync_type="semaphore", id=sem.num, ant_name=sem.name,
                wait_mode="sem-ge-imm", wait_value=value,
            )
        )
        ins.sync_info = mybir.SyncInfo(on_wait=waits, on_update=ups)

    # 1) first consumer of each tile's remote data waits on arrival
    #    (2 increments per peer per broadcast).
    for t, bi in gate_insts:
        splice_wait(bi.ins, rsem[t], 2 * (NC - 1))
    # 2) the sem clears wait on the prelude kernel-entry barrier (an
    #    AllGather inserted at finalize), absorbing launch skew.
    nc._bir_kernel_barrier_sem_replica_groups.extend([set(range(NC))])
    splice_wait(clear0.ins, nc._bir_kernel_barrier_sem,
                nc.bir_kernel_barrier_sem_inc)

    return nc


def _ternary(wmat):
    """Exact {-1,0,1} ternary tensor + fp32 scale, matching the reference."""
    w = np.asarray(wmat, dtype=np.float32)
    s = np.mean(np.abs(w), dtype=np.float32)
    t = np.clip(np.rint(w / (s + np.float32(1e-8))), -1.0, 1.0).astype(np.float32)
    return t, float(s)


_NC_CACHE = {}
_LAST_RESULTS = None


def kernel(**inputs):
    global _LAST_RESULTS
    cfg = CFG_FULL
    L, H, NTOK, NC, TT, VS = (
        cfg["L"], cfg["H"], cfg["NTOK"], cfg["NC"], cfg["TT"], cfg["VS"],
    )
    KT = H // 128
    TPC = TT * 128  # tokens per core
    BF = ml_dtypes.bfloat16
    E4 = ml_dtypes.float8_e4m3

    ids = np.asarray(inputs["input_ids"]).astype(np.int64).reshape(-1)
    embed = np.asarray(inputs["embed"], dtype=np.float32)
    layer_w = np.asarray(inputs["layer_w"], dtype=np.float32)
    layer_b = np.asarray(inputs["layer_b"], dtype=np.float32)
    ln_g = np.asarray(inputs["ln_g"], dtype=np.float32)
    ln_b = np.asarray(inputs["ln_b"], dtype=np.float32)
    final_g = np.asarray(inputs["final_g"], dtype=np.float32)
    final_b = np.asarray(inputs["final_b"], dtype=np.float32)
    head_w = np.asarray(inputs["head_w"], dtype=np.float32)

    trivial = bool(
        np.all(ln_g == 1.0) and np.all(ln_b == 0.0) and np.all(layer_b == 0.0)
        and np.all(final_g == 1.0) and np.all(final_b == 0.0)
    )

    h0_full = embed[ids]  # [NTOK, H] fp32

    scales = []
    wT = np.empty([L, KT, 128, H], dtype=E4)
    for l in range(L):
        t, s = _ternary(layer_w[l])
        scales.append(s)
        wT[l] = np.ascontiguousarray(t.T).reshape(KT, 128, H).astype(E4)
    th, head_scale = _ternary(head_w)
    headT = np.ascontiguousarray(th.T).astype(E4)  # [H, V]

    key = (id(cfg), tuple(scales), head_scale, trivial)
    if key not in _NC_CACHE:
        _NC_CACHE.clear()
        nc = build_nc(cfg, scales, head_scale, trivial)
        nc.finalize()
        _NC_CACHE[key] = nc
    nc = _NC_CACHE[key]

    common = {
        "w": wT,
        "ident": np.eye(128, dtype=BF),
        "eps": np.full((128, 1), EPS, np.float32),
    }
    if not trivial:
        common.update(
            lng=ln_g.astype(BF),
            lnb=ln_b.astype(BF),
            lbias=layer_b.astype(BF),
            fing=final_g.astype(BF),
            finb=final_b.astype(BF),
        )
    in_maps = []
    for c in range(NC):
        mid = np.zeros((128, 8), dtype=BF)
        mid[:, c] = 1.0
        in_maps.append(
            dict(
                common,
                h0=np.ascontiguousarray(
                    h0_full[c * TPC : (c + 1) * TPC].reshape(TT, 128, H)
                ).astype(BF),
                hw=np.ascontiguousarray(
                    headT[:, c * VS : (c + 1) * VS].reshape(KT, 128, VS)
                ),
                mid=mid,
            )
        )

    trace = bool(int(os.environ.get("TRIKERNEL_TRACE", "0")))
    res = run_bass_kernel_spmd(nc, in_maps, core_ids=list(range(NC)), trace=trace)
    _LAST_RESULTS = res

    # unscramble: core r's row block b = j*TT + t holds tokens of sender s
    # (s = r for j=0, else read from the one-hot tag), vocab shard r.
    full = np.empty((NTOK, NC * VS), dtype=np.float32)
    for r in range(NC):
        o = np.asarray(res.results[r]["out"])  # [NTOK, VS] block-rows
        tags = np.asarray(res.results[r]["tags"]).astype(np.float32)
        for j in range(NC):
            for t in range(TT):
                b = j * TT + t
                if j == 0:
                    s = r
                else:
                    s = int(np.argmax(tags[t, 0, j - 1]))
                full[
                    s * TPC + t * 128 : s * TPC + (t + 1) * 128,
                    r * VS : (r + 1) * VS,
                ] = o[b * 128 : (b + 1) * 128]
    return full.reshape(2, 1024, 32000).astype(np.float32)
